# revision 1
# baseline (speedup 1.0000x reference)
"""Trainium2 Bass kernel for nn_MultiHeadAttention_824633721543.

MHA with periodic prefix mask: allowed iff (q % 256) >= (k % 256).
B=2, S=2048, D=768, H=12, Dk=64, WINDOW=256.

Sharding: 8 cores = 2 batches x 4 head-groups (3 heads each). Each core
computes q/k/v projections for its heads, the masked softmax attention, and
a partial O-projection; the host sums the 4 partials per batch and adds bo.

Device-side layout choices (all transpose-free):
  - scores computed as S^T [k,q]: kT slice stationary, qT moving
  - q columns tile-permuted (even 128-tiles | odd 128-tiles) so the mask is:
      even-group x k-lo  -> one shared 128x128 triu tile (0/1 multiply)
      odd-group  x k-lo  -> unmasked
      odd-group  x k-hi  -> shared triu
      even-group x k-hi  -> fully masked, never computed
  - exp on ACT (scale=1/8 folded in), no max-subtraction (scores are small)
  - P@V with [V|1] stationary -> out^T plus denominator row, accumulated over
    the 8 windows in PSUM; normalization via K=1 broadcast matmul + DVE
  - O-projection consumes attn^T directly as the stationary operand
All matmuls run in float32r (fp32 bits, 1 cycle/row at N>=256).
"""

import sys

sys.path.insert(0, "/opt/trn_rl_repo")

import numpy as np

B = 2
S = 2048
D = 768
DK = 64
WIN = 256
NW = S // WIN   # 8 windows
NHC = 3         # heads per core
DH = NHC * DK   # 192
NT = S // 128   # 16 q tiles

_CACHE = {}


def _build_program():
    import concourse.tile as tile
    from concourse import mybir, bacc
    from contextlib import ExitStack

    f32 = mybir.dt.float32
    f32r = mybir.dt.float32r
    Exp = mybir.ActivationFunctionType.Exp
    mult = mybir.AluOpType.mult
    add = mybir.AluOpType.add

    nc = bacc.Bacc("TRN2", target_bir_lowering=False, debug=False)

    xT = nc.dram_tensor("xT", [D, S], f32r, kind="ExternalInput").ap()
    w1 = nc.dram_tensor("w1", [D, 256], f32r, kind="ExternalInput").ap()  # [qh0|qh1|kh0|kh1]
    w2 = nc.dram_tensor("w2", [D, 128], f32r, kind="ExternalInput").ap()  # [qh2|kh2]
    wv = nc.dram_tensor("wv", [D, 256], f32r, kind="ExternalInput").ap()  # WvT pad
    wo = nc.dram_tensor("wo", [DH, D], f32r, kind="ExternalInput").ap()
    btA = nc.dram_tensor("btA", [128, 1], f32, kind="ExternalInput").ap()
    btB = nc.dram_tensor("btB", [128, 1], f32, kind="ExternalInput").ap()
    btC = nc.dram_tensor("btC", [64, 1], f32, kind="ExternalInput").ap()
    btD = nc.dram_tensor("btD", [64, 1], f32, kind="ExternalInput").ap()
    bvb = nc.dram_tensor("bvb", [128, 192], f32, kind="ExternalInput").ap()
    triu = nc.dram_tensor("triu", [128, 128], f32, kind="ExternalInput").ap()
    onesd = nc.dram_tensor("onesd", [128, 64], f32r, kind="ExternalInput").ap()
    out = nc.dram_tensor("out", [S, D], f32, kind="ExternalOutput").ap()

    with tile.TileContext(nc) as tc, ExitStack() as ctx:
        consts = ctx.enter_context(tc.tile_pool(name="consts", bufs=1))
        qkv = ctx.enter_context(tc.tile_pool(name="qkv", bufs=1))

        xtp_cm = tc.tile_pool(name="xtp", bufs=1)
        xtp = xtp_cm.__enter__()
        xT_sb = [xtp.tile([128, S], f32r, tag=f"xt{k}", name=f"xt{k}")
                 for k in range(6)]
        w1_sb = [consts.tile([128, 256], f32r, tag=f"w1_{k}", name=f"w1s{k}")
                 for k in range(6)]
        w2_sb = [consts.tile([128, 128], f32r, tag=f"w2_{k}", name=f"w2s{k}")
                 for k in range(6)]
        wv_sb = [consts.tile([128, 256], f32r, tag=f"wv_{k}", name=f"wvs{k}")
                 for k in range(6)]
        for k in range(6):
            nc.sync.dma_start(out=xT_sb[k], in_=xT[k * 128:(k + 1) * 128, :])
            nc.sync.dma_start(out=w1_sb[k], in_=w1[k * 128:(k + 1) * 128, :])
            nc.sync.dma_start(out=w2_sb[k], in_=w2[k * 128:(k + 1) * 128, :])
            nc.sync.dma_start(out=wv_sb[k], in_=wv[k * 128:(k + 1) * 128, :])
        # Wo^T slice split per head: three [64, 768] tiles (partition base 0)
        wo_sb = [consts.tile([64, D], f32r, tag=f"wo{h}", name=f"wos{h}")
                 for h in range(NHC)]
        for h in range(NHC):
            nc.sync.dma_start(out=wo_sb[h], in_=wo[64 * h:64 * (h + 1), :])
        btA_sb = consts.tile([128, 1], f32, tag="btA")
        btB_sb = consts.tile([128, 1], f32, tag="btB")
        btC_sb = consts.tile([64, 1], f32, tag="btC")
        btD_sb = consts.tile([64, 1], f32, tag="btD")
        nc.sync.dma_start(out=btA_sb, in_=btA)
        nc.sync.dma_start(out=btB_sb, in_=btB)
        nc.sync.dma_start(out=btC_sb, in_=btC)
        nc.sync.dma_start(out=btD_sb, in_=btD)
        bvb_sb = consts.tile([128, 192], f32, tag="bvb")
        nc.sync.dma_start(out=bvb_sb, in_=bvb)
        triu_sb = consts.tile([128, 128], f32, tag="triu")
        nc.sync.dma_start(out=triu_sb, in_=triu)
        ones_row = consts.tile([128, 64], f32r, tag="ones_row")
        nc.sync.dma_start(out=ones_row, in_=onesd)

        # ---- long-lived activation tiles ----
        tileA = qkv.tile([128, S], f32r, tag="tileA")  # [qT_h0|qT_h1], q-permuted
        tileB = qkv.tile([128, S], f32r, tag="tileB")  # [kT_h0|kT_h1], natural
        tileC = qkv.tile([64, S], f32r, tag="tileC")   # qT_h2, permuted
        tileD = qkv.tile([64, S], f32r, tag="tileD")   # kT_h2, natural
        # v natural [s,d] per s-tile: three 65-col groups [V_h | 1]
        v_sb = [qkv.tile([128, 196], f32r, tag=f"v{i}", name=f"vsb{i}")
                for i in range(NT)]
        # attn^T per head, partition base 0
        attnT = [qkv.tile([64, S], f32r, tag=f"attnT{h}", name=f"attnT{h}")
                 for h in range(NHC)]

        def permuted_copy(dst, rows, ps, n, bias):
            """psum 512-span n -> dst cols with even/odd tile permutation."""
            pr3 = ps[0:rows, :].rearrange("p (c two k) -> p c two k", two=2, k=128)
            dr = dst[0:rows, :]
            nc.vector.tensor_scalar_add(
                out=dr[:, 256 * n:256 * n + 256].rearrange("p (c k) -> p c k", k=128),
                in0=pr3[:, :, 0, :], scalar1=bias[0:rows, :])
            nc.vector.tensor_scalar_add(
                out=dr[:, 1024 + 256 * n:1024 + 256 * n + 256].rearrange(
                    "p (c k) -> p c k", k=128),
                in0=pr3[:, :, 1, :], scalar1=bias[0:rows, :])

        # ---- stage A ----
        with tc.tile_pool(name="psA", bufs=2, space="PSUM") as psA:
            for n in range(4):
                xn = [xT_sb[k][:, 512 * n:512 * (n + 1)]
                      for k in range(6)]
                psa = psA.tile([128, 512], f32, tag="psA")
                for k in range(6):
                    nc.tensor.matmul(psa, w1_sb[k][:, 0:128], xn[k],
                                     start=(k == 0), stop=(k == 5))
                permuted_copy(tileA, 128, psa, n, btA_sb)
                psb = psA.tile([128, 512], f32, tag="psA")
                for k in range(6):
                    nc.tensor.matmul(psb, w1_sb[k][:, 128:256], xn[k],
                                     start=(k == 0), stop=(k == 5))
                nc.vector.tensor_scalar_add(
                    out=tileB[:, 512 * n:512 * (n + 1)], in0=psb, scalar1=btB_sb)
                psq = psA.tile([64, 512], f32, tag="psq")
                psk = psA.tile([64, 512], f32, tag="psq")
                for k in range(6):
                    nc.tensor.matmul(psq, w2_sb[k][:, 0:64], xn[k],
                                     start=(k == 0), stop=(k == 5))
                    nc.tensor.matmul(psk, w2_sb[k][:, 64:128], xn[k],
                                     start=(k == 0), stop=(k == 5))
                permuted_copy(tileC, 64, psq, n, btC_sb)
                nc.vector.tensor_scalar_add(
                    out=tileD[:, 512 * n:512 * (n + 1)], in0=psk, scalar1=btD_sb)

            for st in range(NT):
                psv = psA.tile([128, 256], f32, tag="psv")
                for k in range(6):
                    nc.tensor.matmul(
                        psv, xT_sb[k][:, 128 * st:128 * (st + 1)],
                        wv_sb[k], start=(k == 0), stop=(k == 5))
                vt = v_sb[st]
                # copy the 3 heads' 64-col blocks into 65-col groups + bias
                nc.vector.tensor_tensor(
                    out=vt[:, 0:195].rearrange("p (h c) -> p h c", c=65)[:, :, 0:64],
                    in0=psv[:, 0:192].rearrange("p (h c) -> p h c", c=64),
                    in1=bvb_sb.rearrange("p (h c) -> p h c", c=64), op=add)
                # ones columns at 64, 129, 194
                nc.vector.tensor_copy(
                    out=vt[:, 0:195].rearrange("p (h c) -> p h c", c=65)[:, :, 64:65],
                    in_=ones_row[:, 0:3].unsqueeze(2))

        xtp_cm.__exit__(None, None, None)

        # ---- stage B ----
        heads = [
            dict(q=(tileA, 0), k=(tileB, 0)),
            dict(q=(tileA, 64), k=(tileB, 64)),
            dict(q=(tileC, 0), k=(tileD, 0)),
        ]
        triu_b = triu_sb.unsqueeze(1).broadcast_to([128, 8, 128])

        with tc.tile_pool(name="pt", bufs=6) as pt_pool, \
             tc.tile_pool(name="sc", bufs=2, space="PSUM") as sc_pool, \
             tc.tile_pool(name="po", bufs=2, space="PSUM") as out_pool, \
             tc.tile_pool(name="nrm", bufs=2) as nrm_pool:
            for h in range(NHC):
                hd = heads[h]
                qt, qoff = hd["q"]
                kt, koff = hd["k"]
                qv = qt[qoff:qoff + 64, :]
                kv = kt[koff:koff + 64, :]

                for grp in range(2):  # 0=even q-tiles, 1=odd
                    qcols = qv[:, 1024 * grp:1024 * (grp + 1)]
                    po = out_pool.tile([128, 1024], f32, tag="po")
                    state = {"first": [True, True]}

                    def pv_mm(vtile, pt, last):
                        vsl = vtile[:, 65 * h:65 * h + 65]  # [V_h | 1]
                        for sub in range(2):
                            nc.tensor.matmul(
                                po[0:65, 512 * sub:512 * (sub + 1)],
                                vsl,
                                pt[:, 512 * sub:512 * (sub + 1)],
                                start=state["first"][sub], stop=last)
                            state["first"][sub] = False

                    def scores_exp(kblk, mask, mask_eng="dve"):
                        sc = sc_pool.tile([128, 1024], f32, tag="sc")
                        for sub in range(2):
                            nc.tensor.matmul(
                                sc[:, 512 * sub:512 * (sub + 1)], kblk,
                                qcols[:, 512 * sub:512 * (sub + 1)],
                                start=True, stop=True)
                        pt = pt_pool.tile([128, 1024], f32r, tag="pt")
                        nc.scalar.activation(out=pt, in_=sc, func=Exp, scale=0.125)
                        if mask:
                            p3 = pt.rearrange("p (c k) -> p c k", k=128)
                            eng = nc.vector if mask_eng == "dve" else nc.gpsimd
                            eng.tensor_mul(out=p3, in0=p3, in1=triu_b)
                        return pt

                    for w in range(NW):
                        klo = kv[:, WIN * w:WIN * w + 128]
                        if grp == 0:
                            pt = scores_exp(klo, mask=True)
                            pv_mm(v_sb[2 * w], pt, last=(w == NW - 1))
                        else:
                            khi = kv[:, WIN * w + 128:WIN * w + 256]
                            ptlo = scores_exp(klo, mask=False)
                            pthi = scores_exp(khi, mask=True, mask_eng="gpsimd")
                            pv_mm(v_sb[2 * w], ptlo, last=False)
                            pv_mm(v_sb[2 * w + 1], pthi, last=(w == NW - 1))

                    # normalization: denom row 64 -> bcast -> recip -> mul
                    den_sb = nrm_pool.tile([128, 1024], f32r, tag="den")
                    nc.vector.tensor_copy(out=den_sb[64:65, :], in_=po[64:65, :])
                    rec_ps = sc_pool.tile([128, 1024], f32, tag="sc")
                    for sub in range(2):
                        nc.tensor.matmul(
                            rec_ps[0:64, 512 * sub:512 * (sub + 1)],
                            ones_row[64:65, :],
                            den_sb[64:65, 512 * sub:512 * (sub + 1)],
                            start=True, stop=True)
                    rec_sb = nrm_pool.tile([128, 1024], f32, tag="rec")
                    nc.vector.reciprocal_approx_fast(
                        out=rec_sb[0:64, :], in_=rec_ps[0:64, :])
                    nc.vector.tensor_tensor(
                        out=attnT[h][:, 1024 * grp:1024 * (grp + 1)],
                        in0=po[0:64, :], in1=rec_sb[0:64, :], op=mult)

        # ---- stage C ----
        with tc.tile_pool(name="oc", bufs=3, space="PSUM") as oc_pool, \
             tc.tile_pool(name="ost", bufs=3) as ost_pool:
            for p in range(NT):
                pso = oc_pool.tile([128, D], f32, tag="pso")
                for (n0, n1) in ((0, 512), (512, 768)):
                    for h in range(NHC):
                        nc.tensor.matmul(
                            pso[:, n0:n1],
                            attnT[h][:, 128 * p:128 * (p + 1)],
                            wo_sb[h][:, n0:n1],
                            start=(h == 0), stop=(h == NHC - 1))
                ot = ost_pool.tile([128, D], f32, tag="ot")
                nc.scalar.copy(out=ot, in_=pso)
                t = 2 * p if p < 8 else 2 * (p - 8) + 1
                nc.sync.dma_start(out=out[128 * t:128 * (t + 1), :], in_=ot)

    nc.compile()
    return nc


def _prep_core_inputs(inputs, c):
    x = inputs["x"]
    Wq, bq = inputs["Wq"], inputs["bq"]
    Wk, bk = inputs["Wk"], inputs["bk"]
    Wv, bv = inputs["Wv"], inputs["bv"]
    Wo = inputs["Wo"]
    b = c // 4
    r0 = (c % 4) * DH  # first feature row of this core's 192-row head block

    xT = np.ascontiguousarray(np.asarray(x[b]).T.astype(np.float32))
    W1 = np.ascontiguousarray(np.concatenate(
        [Wq[r0:r0 + 128].T, Wk[r0:r0 + 128].T], axis=1).astype(np.float32))
    W2 = np.ascontiguousarray(np.concatenate(
        [Wq[r0 + 128:r0 + 192].T, Wk[r0 + 128:r0 + 192].T], axis=1).astype(np.float32))
    Wvp = np.zeros((D, 256), np.float32)
    Wvp[:, 0:192] = Wv[r0:r0 + 192].T
    wo = np.ascontiguousarray(Wo[:, r0:r0 + 192].T.astype(np.float32))

    return dict(
        xT=xT, w1=W1, w2=W2, wv=Wvp, wo=wo,
        btA=np.ascontiguousarray(bq[r0:r0 + 128].reshape(128, 1).astype(np.float32)),
        btB=np.ascontiguousarray(bk[r0:r0 + 128].reshape(128, 1).astype(np.float32)),
        btC=np.ascontiguousarray(bq[r0 + 128:r0 + 192].reshape(64, 1).astype(np.float32)),
        btD=np.ascontiguousarray(bk[r0 + 128:r0 + 192].reshape(64, 1).astype(np.float32)),
        bvb=np.ascontiguousarray(np.tile(
            bv[r0:r0 + 192].reshape(1, 192), (128, 1)).astype(np.float32)),
        triu=np.ascontiguousarray(np.triu(np.ones((128, 128), np.float32))),
        onesd=np.ones((128, 64), np.float32),
    )


def _install_ntff_hook():
    """Register antenv.axon_hooks with a ctypes NTFF profile hook so
    run_bass_kernel_spmd(trace=True) can capture device-side exec time."""
    import types, ctypes, contextlib, importlib

    try:
        import antenv.axon_hooks  # noqa: F401
        return
    except ImportError:
        pass
    so_path = "/opt/axon/libaxon_pjrt.so"
    lib = ctypes.CDLL(so_path)
    if not hasattr(lib, "axon_start_nrt_profile"):
        return
    lib.axon_start_nrt_profile.argtypes = [
        ctypes.POINTER(ctypes.c_int64), ctypes.c_size_t]
    lib.axon_start_nrt_profile.restype = ctypes.c_int64
    lib.axon_stop_nrt_profile.argtypes = [ctypes.c_char_p]
    lib.axon_stop_nrt_profile.restype = ctypes.c_int64

    @contextlib.contextmanager
    def _hook(output_dir, device_ids):
        import jax
        jax.devices()
        if device_ids:
            ids = (ctypes.c_int64 * len(device_ids))(*device_ids)
            rc = lib.axon_start_nrt_profile(ids, len(device_ids))
        else:
            rc = lib.axon_start_nrt_profile(None, 0)
        if rc != 0:
            raise RuntimeError(f"axon_start_nrt_profile rc={rc}")
        try:
            yield
        finally:
            n = lib.axon_stop_nrt_profile(str(output_dir).encode())
            print(f"profile: {n} file(s) written to {output_dir}")

    mod = types.ModuleType("antenv.axon_hooks")
    mod.get_axon_ntff_profile_hook = lambda: _hook
    mod.set_axon_ntff_profile_hook = lambda h: None
    sys.modules["antenv.axon_hooks"] = mod
    import antenv
    antenv.axon_hooks = mod


def kernel(**inputs):
    import os
    from concourse import bass_utils

    if "nc" not in _CACHE:
        _CACHE["nc"] = _build_program()
    nc = _CACHE["nc"]

    trace = bool(os.environ.get("MHA_TRACE"))
    kwargs = {}
    if trace:
        _install_ntff_hook()
        kwargs = dict(trace=True, tmpdir="/tmp/mha_trace")
        os.makedirs("/tmp/mha_trace", exist_ok=True)

    in_maps = [_prep_core_inputs(inputs, c) for c in range(8)]
    res = bass_utils.run_bass_kernel_spmd(
        nc, in_maps, core_ids=list(range(8)), **kwargs)
    _CACHE["last_results"] = res
    if trace and res.exec_time_ns is not None:
        print(f"HW exec time: {res.exec_time_ns} ns")
    out = np.zeros((B, S, D), np.float32)
    for c in range(8):
        out[c // 4] += res.results[c]["out"]
    out += np.asarray(inputs["bo"], np.float32).reshape(1, 1, D)
    return out



# revision 9
# speedup vs baseline: 1.0222x; 1.0222x over previous
"""Trainium2 Bass kernel for nn_MultiHeadAttention_824633721543.

MHA with periodic prefix mask: allowed iff (q % 256) >= (k % 256).
B=2, S=2048, D=768, H=12, Dk=64, WINDOW=256.

Sharding: 8 cores = 2 batches x 4 head-groups (3 heads each). Each core
computes q/k/v projections for its heads, the masked softmax attention, and
a partial O-projection; the host sums the 4 partials per batch and adds bo.

Device-side layout (all transpose-free, fp16 matmul datapath, f32 PSUM):
  - scores computed as S^T [k,q]: kT slice stationary, qT moving
  - q columns tile-permuted (even 128-tiles | odd 128-tiles) so the mask is:
      even-group x k-lo  -> one shared 128x128 triu tile (0/1 multiply)
      odd-group  x k-lo  -> unmasked
      odd-group  x k-hi  -> shared triu
      even-group x k-hi  -> fully masked, never computed
  - exp on ACT (scale=1/8 folded in, f32 psum in -> fp16 out), no
    max-subtraction (scores are small)
  - P@V with [V|1] stationary -> out^T plus denominator row, accumulated over
    the 8 windows in PSUM; normalization: reciprocal of the denom row,
    K=1 broadcast matmul, DVE multiply
  - O-projection: h0+h1 stacked into one K=128 stationary + h2 at K=64;
    result DMA'd to DRAM directly from PSUM (f32)
"""

import sys

sys.path.insert(0, "/opt/trn_rl_repo")

import numpy as np

B = 2
S = 2048
D = 768
DK = 64
WIN = 256
NW = S // WIN   # 8 windows
NHC = 3         # heads per core
DH = NHC * DK   # 192
NT = S // 128   # 16 q tiles

_CACHE = {}


def _build_program():
    import concourse.tile as tile
    from concourse import mybir, bacc
    from contextlib import ExitStack

    f32 = mybir.dt.float32
    f16 = mybir.dt.float16
    Exp = mybir.ActivationFunctionType.Exp
    Ident = mybir.ActivationFunctionType.Identity
    mult = mybir.AluOpType.mult
    add = mybir.AluOpType.add

    nc = bacc.Bacc("TRN2", target_bir_lowering=False, debug=False)

    xT = nc.dram_tensor("xT", [D, S], f16, kind="ExternalInput").ap()
    w1 = nc.dram_tensor("w1", [D, 256], f16, kind="ExternalInput").ap()  # [qh01|kh01]
    w2 = nc.dram_tensor("w2", [D, 128], f16, kind="ExternalInput").ap()  # [qh2|kh2]
    wv = nc.dram_tensor("wv", [D, 192], f16, kind="ExternalInput").ap()  # WvT
    wo = nc.dram_tensor("wo", [DH, D], f16, kind="ExternalInput").ap()
    btA = nc.dram_tensor("btA", [128, 1], f32, kind="ExternalInput").ap()
    btB = nc.dram_tensor("btB", [128, 1], f32, kind="ExternalInput").ap()
    btCD = nc.dram_tensor("btCD", [128, 1], f32, kind="ExternalInput").ap()
    bvb = nc.dram_tensor("bvb", [128, 192], f32, kind="ExternalInput").ap()
    triu = nc.dram_tensor("triu", [128, 128], f16, kind="ExternalInput").ap()
    onesc = nc.dram_tensor("onesc", [1, 64], f16, kind="ExternalInput").ap()
    out = nc.dram_tensor("out", [S, D], f16, kind="ExternalOutput").ap()

    with tile.TileContext(nc) as tc, ExitStack() as ctx:
        consts = ctx.enter_context(tc.tile_pool(name="consts", bufs=1))
        qkv = ctx.enter_context(tc.tile_pool(name="qkv", bufs=1))

        xT_sb = [qkv.tile([128, S], f16, tag=f"xt{k}", name=f"xt{k}")
                 for k in range(6)]
        w1_sb = [consts.tile([128, 256], f16, tag=f"w1_{k}", name=f"w1s{k}")
                 for k in range(6)]
        w2_sb = [consts.tile([128, 128], f16, tag=f"w2_{k}", name=f"w2s{k}")
                 for k in range(6)]
        wv_sb = [consts.tile([128, 192], f16, tag=f"wv_{k}", name=f"wvs{k}")
                 for k in range(6)]
        for k in range(6):
            nc.sync.dma_start(out=xT_sb[k], in_=xT[k * 128:(k + 1) * 128, :])
            nc.sync.dma_start(out=w1_sb[k], in_=w1[k * 128:(k + 1) * 128, :])
            nc.sync.dma_start(out=w2_sb[k], in_=w2[k * 128:(k + 1) * 128, :])
            nc.sync.dma_start(out=wv_sb[k], in_=wv[k * 128:(k + 1) * 128, :])
        # Wo^T: h0+h1 as one 128-row tile, h2 as 64-row tile
        wo01_sb = consts.tile([128, D], f16, tag="wo01")
        wo2_sb = consts.tile([64, D], f16, tag="wo2")
        nc.sync.dma_start(out=wo01_sb, in_=wo[0:128, :])
        nc.sync.dma_start(out=wo2_sb, in_=wo[128:192, :])
        btA_sb = consts.tile([128, 1], f32, tag="btA")
        btB_sb = consts.tile([128, 1], f32, tag="btB")
        btCD_sb = consts.tile([128, 1], f32, tag="btCD")
        nc.sync.dma_start(out=btA_sb, in_=btA)
        nc.sync.dma_start(out=btB_sb, in_=btB)
        nc.sync.dma_start(out=btCD_sb, in_=btCD)
        bvb_sb = consts.tile([128, 192], f32, tag="bvb")
        nc.sync.dma_start(out=bvb_sb, in_=bvb)
        triu_sb = consts.tile([128, 128], f16, tag="triu")
        nc.sync.dma_start(out=triu_sb, in_=triu)
        ones_row = consts.tile([1, 64], f16, tag="ones_row")
        nc.sync.dma_start(out=ones_row, in_=onesc)

        # ---- long-lived activation tiles ----
        tileA = qkv.tile([128, S], f16, tag="tileA")  # [qT_h0|qT_h1], q-permuted
        tileB = qkv.tile([128, S], f16, tag="tileB")  # [kT_h0|kT_h1], natural
        tileC = qkv.tile([64, S], f16, tag="tileC")   # qT_h2, permuted
        tileD = qkv.tile([64, S], f16, tag="tileD")   # kT_h2, natural
        # v natural [s,d] per s-tile: three 65-col groups [V_h | 1]
        v_sb = [qkv.tile([128, 196], f16, tag=f"v{i}", name=f"vsb{i}")
                for i in range(NT)]
        # attn^T: h0 rows 0:64 + h1 rows 64:128 in one tile; h2 separate
        attnT01 = qkv.tile([128, S], f16, tag="attnT01")
        attnT2 = qkv.tile([64, S], f16, tag="attnT2")

        # ones columns of the v tiles, written once
        for st in range(NT):
            vg = v_sb[st][:, 0:195].rearrange("p (h c) -> p h c", c=65)
            nc.vector.memset(vg[:, :, 64:65], 1.0)

        def permuted_copy(dst, rows, ps, n, bias, eng):
            """psum 512-span n -> dst cols with even/odd tile permutation."""
            pr3 = ps[0:rows, :].rearrange("p (c two k) -> p c two k", two=2, k=128)
            dr = dst[0:rows, :]
            eng.tensor_scalar_add(
                out=dr[:, 256 * n:256 * n + 256].rearrange("p (c k) -> p c k", k=128),
                in0=pr3[:, :, 0, :], scalar1=bias[0:rows, :])
            eng.tensor_scalar_add(
                out=dr[:, 1024 + 256 * n:1024 + 256 * n + 256].rearrange(
                    "p (c k) -> p c k", k=128),
                in0=pr3[:, :, 1, :], scalar1=bias[0:rows, :])

        # ---- stage A ----
        with tc.tile_pool(name="psA", bufs=2, space="PSUM") as psA:
            for n in range(4):
                xn = [xT_sb[k][:, 512 * n:512 * (n + 1)]
                      for k in range(6)]
                psa = psA.tile([128, 512], f32, tag="psA")
                for k in range(6):
                    nc.tensor.matmul(psa, w1_sb[k][:, 0:128], xn[k],
                                     start=(k == 0), stop=(k == 5))
                permuted_copy(tileA, 128, psa, n, btA_sb, nc.vector)
                psb = psA.tile([128, 512], f32, tag="psA")
                for k in range(6):
                    nc.tensor.matmul(psb, w1_sb[k][:, 128:256], xn[k],
                                     start=(k == 0), stop=(k == 5))
                nc.scalar.activation(
                    out=tileB[:, 512 * n:512 * (n + 1)], in_=psb, func=Ident,
                    bias=btB_sb)
                psqk = psA.tile([128, 512], f32, tag="psA")
                for k in range(6):
                    nc.tensor.matmul(psqk, w2_sb[k], xn[k],
                                     start=(k == 0), stop=(k == 5))
                permuted_copy(tileC, 64, psqk, n, btCD_sb, nc.vector)
                nc.scalar.activation(
                    out=tileD[:, 512 * n:512 * (n + 1)], in_=psqk[64:128, :],
                    func=Ident, bias=btCD_sb[64:128, :])

            for st in range(NT):
                psv = psA.tile([128, 192], f32, tag="psv")
                for k in range(6):
                    nc.tensor.matmul(
                        psv, xT_sb[k][:, 128 * st:128 * (st + 1)],
                        wv_sb[k], start=(k == 0), stop=(k == 5))
                vt = v_sb[st]
                nc.vector.tensor_tensor(
                    out=vt[:, 0:195].rearrange("p (h c) -> p h c", c=65)[:, :, 0:64],
                    in0=psv.rearrange("p (h c) -> p h c", c=64),
                    in1=bvb_sb.rearrange("p (h c) -> p h c", c=64), op=add)

        # ---- stage B ----
        heads = [
            dict(q=(tileA, 0), k=(tileB, 0), o=(attnT01, 0)),
            dict(q=(tileA, 64), k=(tileB, 64), o=(attnT01, 64)),
            dict(q=(tileC, 0), k=(tileD, 0), o=(attnT2, 0)),
        ]
        triu_b = triu_sb.unsqueeze(1).broadcast_to([128, 8, 128])

        with tc.tile_pool(name="pt", bufs=6) as pt_pool, \
             tc.tile_pool(name="sc", bufs=2, space="PSUM") as sc_pool, \
             tc.tile_pool(name="po", bufs=2, space="PSUM") as out_pool, \
             tc.tile_pool(name="nrm", bufs=2) as nrm_pool:
            for h in range(NHC):
                hd = heads[h]
                qt, qoff = hd["q"]
                kt, koff = hd["k"]
                ot, ooff = hd["o"]
                qv = qt[qoff:qoff + 64, :]
                kv = kt[koff:koff + 64, :]

                for grp in range(2):  # 0=even q-tiles, 1=odd
                    qcols = qv[:, 1024 * grp:1024 * (grp + 1)]
                    po = out_pool.tile([128, 1024], f32, tag="po")
                    state = {"first": [True, True]}

                    def pv_mm(vtile, pt, last):
                        vsl = vtile[:, 65 * h:65 * h + 65]  # [V_h | 1]
                        for sub in range(2):
                            nc.tensor.matmul(
                                po[0:65, 512 * sub:512 * (sub + 1)],
                                vsl,
                                pt[:, 512 * sub:512 * (sub + 1)],
                                start=state["first"][sub], stop=last)
                            state["first"][sub] = False

                    def scores_exp(kblk, mask):
                        sc = sc_pool.tile([128, 1024], f32, tag="sc")
                        for sub in range(2):
                            nc.tensor.matmul(
                                sc[:, 512 * sub:512 * (sub + 1)], kblk,
                                qcols[:, 512 * sub:512 * (sub + 1)],
                                start=True, stop=True)
                        pt = pt_pool.tile([128, 1024], f16, tag="pt")
                        nc.scalar.activation(out=pt, in_=sc, func=Exp, scale=0.125)
                        if mask:
                            p3 = pt.rearrange("p (c k) -> p c k", k=128)
                            nc.vector.tensor_mul(out=p3, in0=p3, in1=triu_b)
                        return pt

                    for w in range(NW):
                        klo = kv[:, WIN * w:WIN * w + 128]
                        if grp == 0:
                            pt = scores_exp(klo, mask=True)
                            pv_mm(v_sb[2 * w], pt, last=(w == NW - 1))
                        else:
                            khi = kv[:, WIN * w + 128:WIN * w + 256]
                            ptlo = scores_exp(klo, mask=False)
                            pthi = scores_exp(khi, mask=True)
                            pv_mm(v_sb[2 * w], ptlo, last=False)
                            pv_mm(v_sb[2 * w + 1], pthi, last=(w == NW - 1))

                    # normalization: reciprocal of denom row 64 -> bcast -> mul
                    rec_sb = nrm_pool.tile([1, 1024], f16, tag="rec")
                    with nc.allow_low_precision("fp16 attention datapath"):
                        nc.vector.reciprocal(out=rec_sb, in_=po[64:65, :])
                    rec_ps = sc_pool.tile([128, 1024], f32, tag="sc")
                    for sub in range(2):
                        nc.tensor.matmul(
                            rec_ps[0:64, 512 * sub:512 * (sub + 1)],
                            ones_row,
                            rec_sb[:, 512 * sub:512 * (sub + 1)],
                            start=True, stop=True)
                    rec2 = nrm_pool.tile([64, 1024], f16, tag="rec2")
                    nc.vector.tensor_copy(out=rec2, in_=rec_ps[0:64, :])
                    nc.vector.tensor_tensor(
                        out=ot[ooff:ooff + 64, 1024 * grp:1024 * (grp + 1)],
                        in0=po[0:64, :], in1=rec2, op=mult)

        # ---- stage C ----
        with tc.tile_pool(name="oc", bufs=4, space="PSUM") as oc_pool, \
             tc.tile_pool(name="ost", bufs=4) as ost_pool:
            for p in range(NT):
                pso = oc_pool.tile([128, D], f32, tag="pso")
                for (n0, n1) in ((0, 512), (512, 768)):
                    nc.tensor.matmul(
                        pso[:, n0:n1],
                        attnT01[:, 128 * p:128 * (p + 1)],
                        wo01_sb[:, n0:n1], start=True, stop=False)
                    nc.tensor.matmul(
                        pso[:, n0:n1],
                        attnT2[:, 128 * p:128 * (p + 1)],
                        wo2_sb[:, n0:n1], start=False, stop=True)
                ot2 = ost_pool.tile([128, D], f16, tag="ot")
                if p % 2 == 0:
                    nc.vector.tensor_copy(out=ot2, in_=pso)
                else:
                    nc.scalar.copy(out=ot2, in_=pso)
                t = 2 * p if p < 8 else 2 * (p - 8) + 1
                nc.sync.dma_start(out=out[128 * t:128 * (t + 1), :], in_=ot2)

    nc.compile()
    return nc


def _prep_core_inputs(inputs, c):
    x = inputs["x"]
    Wq, bq = inputs["Wq"], inputs["bq"]
    Wk, bk = inputs["Wk"], inputs["bk"]
    Wv, bv = inputs["Wv"], inputs["bv"]
    Wo = inputs["Wo"]
    b = c // 4
    r0 = (c % 4) * DH  # first feature row of this core's 192-row head block

    xT = np.ascontiguousarray(np.asarray(x[b]).T.astype(np.float16))
    W1 = np.ascontiguousarray(np.concatenate(
        [Wq[r0:r0 + 128].T, Wk[r0:r0 + 128].T], axis=1).astype(np.float16))
    W2 = np.ascontiguousarray(np.concatenate(
        [Wq[r0 + 128:r0 + 192].T, Wk[r0 + 128:r0 + 192].T], axis=1).astype(np.float16))
    Wvp = np.ascontiguousarray(Wv[r0:r0 + 192].T.astype(np.float16))
    wo = np.ascontiguousarray(Wo[:, r0:r0 + 192].T.astype(np.float16))

    btCD = np.concatenate([bq[r0 + 128:r0 + 192], bk[r0 + 128:r0 + 192]])
    return dict(
        xT=xT, w1=W1, w2=W2, wv=Wvp, wo=wo,
        btA=np.ascontiguousarray(bq[r0:r0 + 128].reshape(128, 1).astype(np.float32)),
        btB=np.ascontiguousarray(bk[r0:r0 + 128].reshape(128, 1).astype(np.float32)),
        btCD=np.ascontiguousarray(btCD.reshape(128, 1).astype(np.float32)),
        bvb=np.ascontiguousarray(np.tile(
            bv[r0:r0 + 192].reshape(1, 192), (128, 1)).astype(np.float32)),
        triu=np.ascontiguousarray(np.triu(np.ones((128, 128), np.float16))),
        onesc=np.ones((1, 64), np.float16),
    )


def _install_ntff_hook():
    """Register antenv.axon_hooks with a ctypes NTFF profile hook so
    run_bass_kernel_spmd(trace=True) can capture device-side exec time."""
    import types, ctypes, contextlib, importlib

    try:
        import antenv.axon_hooks  # noqa: F401
        return
    except ImportError:
        pass
    so_path = "/opt/axon/libaxon_pjrt.so"
    lib = ctypes.CDLL(so_path)
    if not hasattr(lib, "axon_start_nrt_profile"):
        return
    lib.axon_start_nrt_profile.argtypes = [
        ctypes.POINTER(ctypes.c_int64), ctypes.c_size_t]
    lib.axon_start_nrt_profile.restype = ctypes.c_int64
    lib.axon_stop_nrt_profile.argtypes = [ctypes.c_char_p]
    lib.axon_stop_nrt_profile.restype = ctypes.c_int64

    @contextlib.contextmanager
    def _hook(output_dir, device_ids):
        import jax
        jax.devices()
        if device_ids:
            ids = (ctypes.c_int64 * len(device_ids))(*device_ids)
            rc = lib.axon_start_nrt_profile(ids, len(device_ids))
        else:
            rc = lib.axon_start_nrt_profile(None, 0)
        if rc != 0:
            raise RuntimeError(f"axon_start_nrt_profile rc={rc}")
        try:
            yield
        finally:
            n = lib.axon_stop_nrt_profile(str(output_dir).encode())
            print(f"profile: {n} file(s) written to {output_dir}")

    mod = types.ModuleType("antenv.axon_hooks")
    mod.get_axon_ntff_profile_hook = lambda: _hook
    mod.set_axon_ntff_profile_hook = lambda h: None
    sys.modules["antenv.axon_hooks"] = mod
    import antenv
    antenv.axon_hooks = mod


def kernel(**inputs):
    import os
    from concourse import bass_utils

    if "nc" not in _CACHE:
        _CACHE["nc"] = _build_program()
    nc = _CACHE["nc"]

    trace = bool(os.environ.get("MHA_TRACE"))
    kwargs = {}
    if trace:
        _install_ntff_hook()
        kwargs = dict(trace=True, tmpdir="/tmp/mha_trace")
        os.makedirs("/tmp/mha_trace", exist_ok=True)

    in_maps = [_prep_core_inputs(inputs, c) for c in range(8)]
    res = bass_utils.run_bass_kernel_spmd(
        nc, in_maps, core_ids=list(range(8)), **kwargs)
    _CACHE["last_results"] = res
    if trace and res.exec_time_ns is not None:
        print(f"HW exec time: {res.exec_time_ns} ns")
    out = np.zeros((B, S, D), np.float32)
    for c in range(8):
        out[c // 4] += res.results[c]["out"]
    out += np.asarray(inputs["bo"], np.float32).reshape(1, 1, D)
    return out
